# revision 1
# baseline (speedup 1.0000x reference)
"""Bahdanau attention Trainium2 kernel (8 NeuronCores, batch-parallel).

Math (per batch b):
    e_projT[d, s] = sum_e W_e[e, d] * enc[b, s, e]            (PE, bf16)
    energyT[d, s] = tanh(e_projT + h_proj[b, d] + attn_b[d])  (ACT, bias per-partition)
    scores[s]     = sum_d v[d] * energyT[d, s]                (PE, M=1 matmuls)
    w = softmax(where(mask==0, -inf, scores))                 (row ops)
    context[e]    = sum_s w[s] * enc[b, s, e]                 (PE or DVE)

enc enters the PE with E2 on partitions (contraction dim), so each [128,128]
sub-tile of enc is transposed on-chip via PE transpose-mode matmuls (bf16,
1 cyc/row), packed 4-per-PSUM-bank, then copied to SBUF (DVE/ACT alternating).
A fraction of the transposes can instead ride the DMA xbar (SBUF->SBUF,
2-byte dtype) to offload the PE. All heavy matmuls run in bf16 (fp32 matmul
is 4x slower on TRN2); accumulation is fp32 in PSUM. h_proj is computed once
per core in fp32.
"""

import os
import sys

for _p in ("/opt/trn_rl_repo", "/root/.axon_site/_ro/trn_rl_repo"):
    if os.path.isdir(_p) and _p not in sys.path:
        sys.path.insert(0, _p)

# recover cleanly if a previous session left a core wedged
os.environ.setdefault("NEURON_RT_RESET_CORES", "1")

import numpy as np

import concourse.bass as bass
import concourse.tile as tile
from concourse import bacc, masks, mybir
from concourse._compat import with_exitstack
from concourse.bass_utils import run_bass_kernel_spmd

F32 = mybir.dt.float32
BF16 = mybir.dt.bfloat16
I32 = mybir.dt.int32
AF = mybir.ActivationFunctionType

N_CORES = 8
B, S, E2, D = 64, 1024, 1024, 512
BPC = B // N_CORES  # batches per core
NEG_INF = float(np.finfo(np.float32).min)

ST = S // 128   # 8 s-tiles per batch
ET = E2 // 128  # 8 e-blocks
DT = D // 128   # 4 d-tiles
NBLK = S // 512  # 2 s-halves of 512

# context on "pe" (nat-resident matmuls) or "dve" (wrep + mult/reduce)
CTX_MODE = os.environ.get("K_CTX_MODE", "pe")
# how many of the ET=8 e-blocks' transposes go via DMA xbar instead of PE
T_DMA_BLOCKS = int(os.environ.get("K_T_DMA_BLOCKS", "0"))
# repeat the whole per-core workload R times inside the NEFF (timing harness)
REPEAT = int(os.environ.get("K_REPEAT", "1"))


@with_exitstack
def _attn_kernel(ctx, tc, enc_d, hid_d, mask_d, w_d, b_d, v_d, ctx_d, aw_d):
    nc = tc.nc

    const = ctx.enter_context(tc.tile_pool(name="const", bufs=1))
    natp = ctx.enter_context(tc.tile_pool(name="nat", bufs=3))
    encTp = ctx.enter_context(tc.tile_pool(name="encT", bufs=2))
    enp = ctx.enter_context(tc.tile_pool(name="energy", bufs=10))
    smp = ctx.enter_context(tc.tile_pool(name="small", bufs=6))
    cxp = ctx.enter_context(tc.tile_pool(name="ctxc", bufs=3))

    # ---------------- constants / prologue ----------------
    ident_b = const.tile([128, 128], BF16)
    ident_f = const.tile([128, 128], F32)
    masks.make_identity(nc, ident_b[:])
    masks.make_identity(nc, ident_f[:])

    neginf = const.tile([1, S], F32)
    nc.gpsimd.memset(neginf[:], NEG_INF)
    ones_b = const.tile([1, 128], BF16)
    nc.gpsimd.memset(ones_b[:], 1.0)

    # prefetch the first batch's enc ahead of the bulk weight loads: the
    # transposes need only ident_b + nat, so the PE can start early.
    prenat = {}
    for b in range(min(1, BPC)):
        pn = natp.tile([128, ST * E2], BF16, tag="nat", name=f"nat_pre{b}")
        for t in range(ST):
            nc.gpsimd.dma_start(
                pn[:, t * E2 : (t + 1) * E2], enc_d[b, t * 128 : (t + 1) * 128, :]
            )
        prenat[b] = pn

    # weights as [p, k, d]: row index = k*128+p (2D contiguous DMA per k-slice)
    Wh = const.tile([128, DT, D], F32)
    for k in range(DT):
        nc.sync.dma_start(Wh[:, k, :], w_d[k * 128 : (k + 1) * 128, :])
    We = const.tile([128, ET, D], BF16)
    for k in range(ET):
        nc.gpsimd.dma_start(We[:, k, :], w_d[D + k * 128 : D + (k + 1) * 128, :])

    ab = const.tile([128, DT], F32)  # attn_b as [p, m], d = m*128+p
    nc.sync.dma_start(ab[:], b_d[:].rearrange("(m p) -> p m", p=128))
    vf = const.tile([128, DT], F32)  # v_W as [p, m]
    nc.sync.dma_start(vf[:], v_d[:].rearrange("(m p) -> p m", p=128))
    vb = const.tile([128, DT], BF16)
    nc.vector.tensor_copy(vb[:], vf[:])

    hid = const.tile([BPC, D], F32)
    nc.sync.dma_start(hid[:], hid_d[:])
    mask_t = const.tile([1, BPC * S], I32)
    nc.sync.dma_start(mask_t[:], mask_d[:])

    # hiddenT and h_projT + bias -> hb[p, m, b]
    hb = const.tile([128, DT, BPC], F32)
    hT = const.tile([128, DT, BPC], F32)
    with tc.tile_pool(name="psum_pro", bufs=2, space="PSUM") as psum_pro:
        for k in range(DT):
            tp = psum_pro.tile([128, BPC], F32, tag="pro")
            nc.tensor.transpose(tp[:], hid[:, k * 128 : (k + 1) * 128], ident_f[0:BPC, 0:BPC])
            nc.vector.tensor_copy(hT[:, k, :], tp[:])
        for m in range(DT):
            hp = psum_pro.tile([128, BPC], F32, tag="pro")
            for k in range(DT):
                nc.tensor.matmul(
                    hp[:],
                    Wh[:, k, m * 128 : (m + 1) * 128],
                    hT[:, k, :],
                    start=(k == 0),
                    stop=(k == DT - 1),
                )
            nc.vector.tensor_scalar(
                out=hb[:, m, :], in0=hp[:], scalar1=ab[:, m : m + 1], scalar2=None,
                op0=mybir.AluOpType.add,
            )

    # ---------------- main psum pools ----------------
    psum_t = ctx.enter_context(tc.tile_pool(name="psum_t", bufs=3, space="PSUM"))
    psum_mm = ctx.enter_context(tc.tile_pool(name="psum_mm", bufs=3, space="PSUM"))
    psum_sc = ctx.enter_context(tc.tile_pool(name="psum_sc", bufs=1, space="PSUM"))

    if REPEAT > 1:
        rep_cm = tc.For_i(0, REPEAT)
        rep_cm.__enter__()

    # software pipeline: batch b's context phase (which waits on b's softmax)
    # is emitted after batch b+1's transposes + e_proj, so the PE never
    # stalls on the softmax chain in program order.
    pending_ctx = []

    def emit_ctx(state):
        b, aw, wb, nat, encT = state
        if CTX_MODE == "pe":
            # w as columns [p, t]: 8 tiny PE transposes of [1,128] segments
            wcp = psum_sc.tile([128, ST], F32, tag="cps")
            for t in range(ST):
                nc.tensor.transpose(
                    wcp[:, t : t + 1], aw[:, t * 128 : (t + 1) * 128], ident_f[0:1, 0:1]
                )
            wcol = cxp.tile([128, ST], BF16, tag="wcol")
            nc.vector.tensor_copy(wcol[:], wcp[:])
            # contextT[1, e] = sum_t w_col[:,t].T @ nat[s-tile t, e-range]
            ctx_row = cxp.tile([1, E2], F32, tag="ctx_row")
            for half in range(2):
                cps = psum_sc.tile([1, 512], F32, tag="cps")
                for t in range(ST):
                    nc.tensor.matmul(
                        cps[:],
                        wcol[:, t : t + 1],
                        nat[:, t * E2 + half * 512 : t * E2 + half * 512 + 512],
                        start=(t == 0),
                        stop=(t == ST - 1),
                    )
                nc.vector.tensor_copy(ctx_row[:, half * 512 : (half + 1) * 512], cps[:])
            nc.sync.dma_start(ctx_d[b : b + 1, :], ctx_row[:])
        else:
            wrep = cxp.tile([128, S], BF16, tag="wrep")
            for blk in range(NBLK):
                wp = psum_mm.tile([128, 512], F32, tag="mm")
                nc.tensor.matmul(wp[:], ones_b[:], wb[:, blk * 512 : (blk + 1) * 512])
                nc.vector.tensor_copy(wrep[:, blk * 512 : (blk + 1) * 512], wp[:])
            ctx_cols = cxp.tile([128, ET], F32, tag="ctx_cols")
            for j in range(ET):
                junk = cxp.tile([128, S], BF16, tag="junk")
                nc.vector.tensor_tensor(
                    out=junk[:], in0=encT[:, j * S : (j + 1) * S], in1=wrep[:],
                    op=mybir.AluOpType.mult,
                )
                nc.vector.reduce_sum(
                    ctx_cols[:, j : j + 1], junk[:], axis=mybir.AxisListType.X
                )
            cps2 = psum_sc.tile([ET, 128], F32, tag="cps2")
            nc.tensor.transpose(cps2[:], ctx_cols[:], ident_f[:])
            ctx_row2 = cxp.tile([ET, 128], F32, tag="ctx_row2")
            nc.vector.tensor_copy(ctx_row2[:], cps2[:])
            nc.sync.dma_start(ctx_d[b].rearrange("(p f) -> p f", p=ET), ctx_row2[:])

    for b in range(BPC):
        # load enc[b] with cast to bf16: nat[p, t*E2 + e] = enc[b, t*128+p, e]
        if b in prenat:
            nat = prenat[b]
        else:
            nat = natp.tile([128, ST * E2], BF16, tag="nat")
            for t in range(ST):
                nc.gpsimd.dma_start(
                    nat[:, t * E2 : (t + 1) * E2], enc_d[b, t * 128 : (t + 1) * 128, :]
                )

        # transpose to encT[p, j*S + s] = enc[b, s, j*128+p]
        encT = encTp.tile([128, ET * S], BF16)
        cp_eng = 0
        for j in range(ET):
            if j < T_DMA_BLOCKS:
                # DMA xbar transpose, one [128s,128e] -> [128e,128s] tile at a time
                for t in range(ST):
                    nc.sync.dma_start(
                        encT[:, j * S + t * 128 : j * S + (t + 1) * 128],
                        nat[:, t * E2 + j * 128 : t * E2 + (j + 1) * 128],
                        transpose=True,
                    )
                continue
            for h in range(NBLK):
                tp = psum_t.tile([128, 512], BF16, tag="tp")
                for q in range(4):
                    t = 4 * h + q
                    nc.tensor.transpose(
                        tp[:, q * 128 : (q + 1) * 128],
                        nat[:, t * E2 + j * 128 : t * E2 + (j + 1) * 128],
                        ident_b[:],
                    )
                dst = encT[:, j * S + h * 512 : j * S + (h + 1) * 512]
                if cp_eng == 0:
                    nc.vector.tensor_copy(dst, tp[:])
                else:
                    nc.scalar.copy(dst, tp[:])
                cp_eng ^= 1

        # e_projT -> tanh (all blocks first, then scores: gives ACT slack)
        ens = {}
        for blk in range(NBLK):
            for m in range(DT):
                mm = psum_mm.tile([128, 512], F32, tag="mm")
                for k in range(ET):
                    nc.tensor.matmul(
                        mm[:],
                        We[:, k, m * 128 : (m + 1) * 128],
                        encT[:, k * S + blk * 512 : k * S + blk * 512 + 512],
                        start=(k == 0),
                        stop=(k == ET - 1),
                    )
                en = enp.tile([128, 512], BF16, tag="en")
                nc.scalar.activation(en[:], mm[:], AF.Tanh, bias=hb[:, m, b : b + 1])
                ens[(blk, m)] = en
        scores = smp.tile([1, S], F32, tag="row_f32")
        for blk in range(NBLK):
            sc = psum_sc.tile([1, 512], F32, tag="sc")
            for m in range(DT):
                nc.tensor.matmul(
                    sc[:], vb[:, m : m + 1], ens[(blk, m)][:],
                    start=(m == 0), stop=(m == DT - 1),
                )
            nc.vector.tensor_copy(scores[:, blk * 512 : (blk + 1) * 512], sc[:])

        # mask: out = where(mask != 0, scores, -inf)
        masked = smp.tile([1, S], F32, tag="row_f32")
        nc.vector.tensor_copy(masked[:], neginf[:])
        nc.vector.copy_predicated(masked[:], mask_t[:, b * S : (b + 1) * S], scores[:])

        # softmax on the [1, S] row
        nmax = smp.tile([1, 1], F32, tag="one")
        nc.vector.reduce_max(nmax[:], masked[:], axis=mybir.AxisListType.X, negate=True)
        expw = smp.tile([1, S], F32, tag="row_f32")
        sume = smp.tile([1, 1], F32, tag="one")
        nc.scalar.activation(expw[:], masked[:], AF.Exp, bias=nmax[:, 0:1], accum_out=sume[:])
        rcp = smp.tile([1, 1], F32, tag="one")
        nc.vector.reciprocal(rcp[:], sume[:])
        aw = smp.tile([1, S], F32, tag="row_f32")
        nc.vector.tensor_scalar_mul(aw[:], expw[:], rcp[:, 0:1])
        nc.sync.dma_start(aw_d[b : b + 1, :], aw[:])

        wb = smp.tile([1, S], BF16, tag="row_bf16")
        nc.vector.tensor_copy(wb[:], aw[:])

        pending_ctx.append((b, aw, wb, nat, encT))
        if len(pending_ctx) > 1:
            emit_ctx(pending_ctx.pop(0))

    while pending_ctx:
        emit_ctx(pending_ctx.pop(0))

    if REPEAT > 1:
        rep_cm.__exit__(None, None, None)


def build():
    nc = bacc.Bacc("TRN2", target_bir_lowering=False, debug=False)
    enc_d = nc.dram_tensor("enc", [BPC, S, E2], F32, kind="ExternalInput")
    hid_d = nc.dram_tensor("hidden", [BPC, D], F32, kind="ExternalInput")
    mask_d = nc.dram_tensor("mask", [BPC, S], I32, kind="ExternalInput")
    w_d = nc.dram_tensor("attn_w", [E2 + D, D], F32, kind="ExternalInput")
    b_d = nc.dram_tensor("attn_b", [D], F32, kind="ExternalInput")
    v_d = nc.dram_tensor("v_w", [D], F32, kind="ExternalInput")
    ctx_d = nc.dram_tensor("ctx_out", [BPC, E2], F32, kind="ExternalOutput")
    aw_d = nc.dram_tensor("aw_out", [BPC, S], F32, kind="ExternalOutput")

    with tile.TileContext(nc) as tc:
        _attn_kernel(tc, enc_d, hid_d, mask_d, w_d, b_d, v_d, ctx_d, aw_d)
    nc.compile()
    return nc


_NC_CACHE = None


def _get_nc():
    global _NC_CACHE
    if _NC_CACHE is None:
        _NC_CACHE = build()
    return _NC_CACHE


def _make_in_maps(hidden, encoder_outputs, mask, attn_W, attn_b, v_W):
    in_maps = []
    for c in range(N_CORES):
        sl = slice(c * BPC, (c + 1) * BPC)
        in_maps.append(
            {
                "enc": np.ascontiguousarray(encoder_outputs[sl]),
                "hidden": np.ascontiguousarray(hidden[sl]),
                "mask": np.ascontiguousarray(mask[sl]),
                "attn_w": attn_W,
                "attn_b": attn_b,
                "v_w": v_W,
            }
        )
    return in_maps


def run(hidden, encoder_outputs, mask, attn_W, attn_b, v_W, trace=False):
    """Run the bass kernel; returns ((context, attn_weights), BassKernelResults)."""
    nc = _get_nc()
    in_maps = _make_in_maps(hidden, encoder_outputs, mask, attn_W, attn_b, v_W)
    res = run_bass_kernel_spmd(nc, in_maps, list(range(N_CORES)), trace=trace)
    context = np.concatenate([res.results[c]["ctx_out"] for c in range(N_CORES)], axis=0)
    attn_w = np.concatenate([res.results[c]["aw_out"] for c in range(N_CORES)], axis=0)
    return (context, attn_w), res


def kernel(hidden, encoder_outputs, mask, attn_W, attn_b, v_W):
    (context, attn_w), _ = run(
        np.asarray(hidden, dtype=np.float32),
        np.asarray(encoder_outputs, dtype=np.float32),
        np.asarray(mask, dtype=np.int32),
        np.asarray(attn_W, dtype=np.float32),
        np.asarray(attn_b, dtype=np.float32),
        np.asarray(v_W, dtype=np.float32),
    )
    return context, attn_w


if __name__ == "__main__":
    nc = build()
    n_inst = sum(len(bb.instructions) for f in nc.m.functions for bb in f.blocks)
    print("build OK, instructions:", n_inst)

